# revision 12
# baseline (speedup 1.0000x reference)
"""MHC-lite block kernel for 8x TRN2 NeuronCores.

The wall-clock of a device call in this environment is dominated by
host<->device transfer over the axon tunnel (~43MB/s each direction,
partial duplex, ~80ms dispatch latency, ~50ms fixed cost per transfer),
so the design minimizes steady-state bytes on the wire:

  - host (f32): rms-norm stats, the three small projections (2 GFLOP),
    gates/softmax/H, layer_input (li), and the final mixed+expanded
    combine against the original f32 x.
  - device: FFN only (99% of model FLOPs), int8-in/int8-out.

Steady-state per-call traffic: 8.4MB li int8 h2d + 8.4MB y int8 d2h.
Everything else is off the per-call path:
  - FFN weights ship once (int8, 1/8 shard per core + on-device
    AllGather) and stay resident as a jax device array, keyed by a
    fingerprint of the incoming weights.
  - The donated output buffer is generated on-device (jnp.zeros under
    jit) instead of being transferred.
  - The PJRT wrapper (shard_map over _bass_exec_p) is jitted once and
    cached; per-call work is pure dispatch.

Quantization: w1,w2 int8 with global host scales s1,s2; s1 folds into
the per-token li dequant scale, s2 applies on host to the returned
per-token output scales. gelu at true scale, f32 PSUM; output re-quant
per token. f32 scales ride in the last rows of the int8 tensors via
bitcast APs.
"""

import numpy as np
import zlib

try:  # cache the XLA wrapper compile across processes
    import jax as _jax
    _jax.config.update("jax_compilation_cache_dir", "/tmp/jax_cache")
    _jax.config.update("jax_persistent_cache_min_compile_time_secs", 0.0)
    _jax.config.update("jax_persistent_cache_min_entry_size_bytes", -1)
except Exception:
    pass

import jax
import jax.numpy as jnp
from jax.sharding import Mesh, PartitionSpec, NamedSharding

import concourse.bacc as bacc
import concourse.mybir as mybir
import concourse.tile as tile
from concourse import bass2jax

try:
    from jax.experimental.shard_map import shard_map
except ImportError:
    from jax.shard_map import shard_map  # newer jax

N_CORES = 8
T_CORE = 1024          # tokens per core
NTOK = 8192            # total tokens
HID = 1024
NCH = 4096
DFF = 4096
EPS = 1.1920929e-07

LI_ROWS = T_CORE + 4       # 1024 li rows + 4 rows f32 per-token scales
W_ROWS = 2 * DFF + 32      # full w1 (4096) + full w2 (4096) + 32 rows b1 f32
Y_ROWS = T_CORE + 4        # 1024 out rows + 4 rows f32 per-token scales

F32 = mybir.dt.float32
BF16 = mybir.dt.bfloat16
I8 = mybir.dt.int8

_CACHE = {}


def _build_module():
    nc = bacc.Bacc("TRN2", target_bir_lowering=False, debug=False,
                   num_devices=1)
    li8_d = nc.dram_tensor("li8", [LI_ROWS, 1024], I8,
                           kind="ExternalInput").ap()
    wblob_d = nc.dram_tensor("wblob", [W_ROWS, 1024], I8,
                             kind="ExternalInput").ap()
    y8_d = nc.dram_tensor("y8", [Y_ROWS, 1024], I8, kind="ExternalOutput").ap()

    with tile.TileContext(nc, trace_sim=False) as tc:
        _emit(nc, tc, li8_d, wblob_d, y8_d)
    nc.compile()
    return nc


def _emit(nc, tc, li8_d, wblob_d, y8_d):
    pools = []

    def _pool(*a, **k):
        p = tc.alloc_tile_pool(*a, **k)
        pools.append(p)
        return p

    ysc_view = y8_d[T_CORE:T_CORE + 4, :].bitcast(F32) \
        .rearrange("a (t o) -> (a t) o", t=256)          # [1024, 1] f32

    # Full weights live in wblob (device-resident across calls; uploaded
    # once). No collective: keeps per-core executions independent so the
    # d2h of early cores overlaps the h2d of later cores.
    def _w1g_rows(m):                        # dff tile m of w1
        return wblob_d[m * 128:(m + 1) * 128, :]

    def _w2g_rows(m):                        # dff tile m of w2
        return wblob_d[DFF + m * 128:DFF + (m + 1) * 128, :]

    cp = _pool(name="const", bufs=1)
    w1_sb = cp.tile([128, 32 * HID], BF16, tag="w1sb")   # raw int values
    w2_sb = cp.tile([128, 32 * HID], BF16, tag="w2sb")   # raw int values
    f_sb = cp.tile([128, 64], F32, tag="fsb")            # b1 | pad
    lsc_sb = cp.tile([128, 8], F32, tag="lscsb")         # li scales, s1 folded
    lit_sb = cp.tile([128, 8 * T_CORE], BF16, tag="lit")  # li^T

    bview = wblob_d[2 * DFF:2 * DFF + 32, :].bitcast(F32) \
        .rearrange("a (p c) -> (a p) c", p=4)            # [128, 64] f32
    nc.sync.dma_start(f_sb[:, :], bview[:, :])
    b1_sb = f_sb[:, 0:32]
    lview = li8_d[T_CORE:T_CORE + 4, :].bitcast(F32) \
        .rearrange("a (p c) -> (a p) c", p=32)           # [128, 8] f32
    nc.sync.dma_start(lsc_sb[:, :], lview[:, :])

    stp = _pool(name="stage", bufs=3)
    for m in range(32):
        st1 = stp.tile([128, HID], I8, tag="w1st")
        nc.sync.dma_start(st1[:, :], _w1g_rows(m))
        nc.vector.tensor_copy(w1_sb[:, m * HID:(m + 1) * HID], st1[:, :])
        st2 = stp.tile([128, HID], I8, tag="w2st")
        nc.sync.dma_start(st2[:, :], _w2g_rows(m))
        nc.vector.tensor_copy(w2_sb[:, m * HID:(m + 1) * HID], st2[:, :])

    # li: load int8 token-major, dequant (scale includes s1), transpose
    lqp = _pool(name="lq", bufs=3)
    lbp = _pool(name="lb", bufs=3)
    for r in range(8):                       # token tiles
        li8 = lqp.tile([128, HID], I8, tag="li8")
        nc.sync.dma_start(li8[:, :], li8_d[r * 128:(r + 1) * 128, :])
        libf = lbp.tile([128, HID], BF16, tag="libf")
        nc.vector.tensor_scalar_mul(libf[:, :], li8[:, :],
                                    lsc_sb[:, r:r + 1])
        for k in range(8):                   # hid tiles -> transpose
            nc.sync.dma_start_transpose(
                lit_sb[:, k * T_CORE + r * 128: k * T_CORE + r * 128 + 128],
                libf[:, k * 128:(k + 1) * 128])

    hp = _pool(name="h", bufs=4)
    qp = _pool(name="q", bufs=4)
    scp = _pool(name="sc", bufs=4)
    psA = _pool(name="psA", bufs=4, space="PSUM")
    psB = _pool(name="psB", bufs=4, space="PSUM")

    # mm2 for step m is emitted DELTA mm1-blocks later so the PE never
    # blocks on the just-issued gelu (~80us per blocking wait otherwise).
    DELTA = 3

    for g in range(4):                       # groups of 256 tokens
        t0 = g * 256
        fps = [[psA.tile([128, 512], F32, tag="psA",
                         name=f"fps_{g}_{ti}_{hf}")
                for hf in range(2)] for ti in range(2)]
        hs = {}

        def _mm2(m):
            h_src = hs.pop(m)
            for ti in range(2):
                for hf in range(2):
                    nc.tensor.matmul(
                        fps[ti][hf][:, :],
                        h_src[:, ti * 128:(ti + 1) * 128],
                        w2_sb[:, m * HID + hf * 512: m * HID + hf * 512 + 512],
                        start=(m == 0), stop=(m == 31))

        for m in range(32):                  # dff tiles of 128
            hmp = psB.tile([128, 256], F32, tag="psB")
            for k in range(8):               # hid contraction tiles
                nc.tensor.matmul(hmp[:, :],
                                 w1_sb[:, m * HID + k * 128:
                                       m * HID + k * 128 + 128],
                                 lit_sb[:, k * T_CORE + t0:
                                        k * T_CORE + t0 + 256],
                                 start=(k == 0), stop=(k == 7))
            h_m = hp.tile([128, 256], BF16, tag="h", name=f"h_{g}_{m}")
            nc.scalar.activation(h_m[:, :], hmp[:, :],
                                 mybir.ActivationFunctionType.Gelu_apprx_tanh,
                                 bias=b1_sb[:, m:m + 1])
            hs[m] = h_m
            if m >= DELTA:
                _mm2(m - DELTA)
        for m in range(32 - DELTA, 32):
            _mm2(m)
        for ti in range(2):
            # per-token absmax over the 1024 output cols (raw 1/s2 scale)
            sc = scp.tile([128, 8], F32, tag="sc")
            nc.vector.reduce_max(sc[:, 0:1], fps[ti][0][:, :],
                                 axis=mybir.AxisListType.X,
                                 apply_absolute_value=True)
            nc.vector.reduce_max(sc[:, 1:2], fps[ti][1][:, :],
                                 axis=mybir.AxisListType.X,
                                 apply_absolute_value=True)
            nc.vector.tensor_max(sc[:, 2:3], sc[:, 0:1], sc[:, 1:2])
            nc.scalar.activation(sc[:, 3:4], sc[:, 2:3],
                                 mybir.ActivationFunctionType.Copy,
                                 bias=1e-12)
            nc.vector.reciprocal(sc[:, 4:5], sc[:, 3:4])
            nc.scalar.activation(sc[:, 5:6], sc[:, 4:5],
                                 mybir.ActivationFunctionType.Copy,
                                 scale=127.0)                  # inv
            nc.scalar.activation(sc[:, 6:7], sc[:, 3:4],
                                 mybir.ActivationFunctionType.Copy,
                                 scale=1.0 / 127.0)            # out scale
            nc.sync.dma_start(ysc_view[t0 + ti * 128: t0 + ti * 128 + 128, :],
                              sc[:, 6:7])
            for hf in range(2):
                q = qp.tile([128, 512], I8, tag="q")
                nc.vector.tensor_scalar_mul(q[:, :], fps[ti][hf][:, :],
                                            sc[:, 5:6])
                nc.sync.dma_start(
                    y8_d[t0 + ti * 128: t0 + ti * 128 + 128,
                         hf * 512:(hf + 1) * 512],
                    q[:, :])

    for p in reversed(pools):
        p.release()


# ---------------------------------------------------------------------------
# PJRT runner: jitted once, reused every call.
# ---------------------------------------------------------------------------

def _build_runner(nc):
    bass2jax.install_neuronx_cc_hook()
    partition_name = nc.partition_id_tensor.name if nc.partition_id_tensor \
        else None
    in_names, out_names, out_avals, zero_shapes = [], [], [], []
    for alloc in nc.m.functions[0].allocations:
        if not isinstance(alloc, mybir.MemoryLocationSet):
            continue
        name = alloc.memorylocations[0].name
        if alloc.kind == "ExternalInput":
            if name != partition_name:
                in_names.append(name)
        elif alloc.kind == "ExternalOutput":
            shape = tuple(alloc.tensor_shape)
            dtype = mybir.dt.np(alloc.dtype)
            out_names.append(name)
            out_avals.append(jax.core.ShapedArray(shape, dtype))
            zero_shapes.append((shape, dtype))
    n_params = len(in_names)
    n_outs = len(out_avals)
    in_names_all = list(in_names) + list(out_names)
    if partition_name is not None:
        in_names_all.append(partition_name)

    def _body(*args):
        operands = list(args)
        if partition_name is not None:
            operands.append(bass2jax.partition_id_tensor())
        outs = bass2jax._bass_exec_p.bind(
            *operands, out_avals=tuple(out_avals),
            in_names=tuple(in_names_all), out_names=tuple(out_names),
            lowering_input_output_aliases=(), sim_require_finite=True,
            sim_require_nnan=True, nc=nc)
        return tuple(outs)

    devices = jax.devices()[:N_CORES]
    mesh = Mesh(np.asarray(devices), ("core",))
    shard = NamedSharding(mesh, PartitionSpec("core"))
    donate = tuple(range(n_params, n_params + n_outs))
    # one per-device jit: calls with device-c-committed args execute on c,
    # independently — core 0's output d2h overlaps later cores' h2d.
    run1 = jax.jit(_body, donate_argnums=donate, keep_unused=True)
    zshape, zdtype = zero_shapes[0]
    from jax.sharding import SingleDeviceSharding
    mkz = [jax.jit(lambda zs=zshape, zd=zdtype: jnp.zeros(zs, zd),
                   out_shardings=SingleDeviceSharding(d)) for d in devices]
    return dict(run1=run1, mkz=mkz, shard=shard, in_names=in_names,
                devices=devices)


def get_runner():
    if "runner" not in _CACHE:
        if "nc" not in _CACHE:
            _CACHE["nc"] = _build_module()
        _CACHE["runner"] = _build_runner(_CACHE["nc"])
    return _CACHE["runner"]


def _run_device(li_global):
    """li_global: [8*LI_ROWS, 1024] int8 -> y: [8, Y_ROWS, 1024] int8."""
    r = get_runner()
    xg = jax.device_put(li_global, r["shard"])
    shards = sorted(xg.addressable_shards, key=lambda s: s.device.id)
    wdev = _CACHE["wdev"]
    ys = []
    for c in range(N_CORES):
        z = r["mkz"][c]()
        (y,) = r["run1"](shards[c].data, wdev[c], z)
        y.copy_to_host_async()
        ys.append(y)
    out = np.empty((N_CORES, Y_ROWS, 1024), np.int8)
    for c in range(N_CORES):
        out[c] = np.asarray(ys[c])
    return out


# ---------------------------------------------------------------------------
# Host-side prep (jax on the XLA CPU backend — numpy here links reference
# BLAS and is ~6x slower on the elementwise-heavy combine)
# ---------------------------------------------------------------------------

_CPU = jax.devices("cpu")[0]


@jax.jit
def _prep_jit(x, wcat, bcat, a_pre, a_post, a_res, perm):
    ssq = jnp.einsum('tc,tc->t', x, x)
    s = jax.lax.rsqrt(ssq / NCH + EPS)
    z = (x @ wcat) * s[:, None] + bcat                     # [NTOK, 32]
    h_pre = jax.nn.sigmoid(a_pre * z[:, 0:4])
    h_post = 2.0 * jax.nn.sigmoid(a_post * z[:, 4:8])
    a_soft = jax.nn.softmax(a_res * z[:, 8:32], axis=-1)
    H = a_soft @ perm                                      # [NTOK, 16]
    x4 = x.reshape(NTOK, 4, HID)
    li = jnp.einsum('tn,tnc->tc', h_pre, x4)
    rowmax = jnp.abs(li).max(axis=1) + 1e-30
    li_q = jnp.clip(jnp.round(li * (127.0 / rowmax[:, None])),
                    -127, 127).astype(jnp.int8)
    return li_q, rowmax, li, h_post, H


@jax.jit
def _combine_jit(q, ysc, li, h_post, H, x4, s2b2):
    ffn = q.astype(jnp.float32) * (ysc * s2b2[0]) + s2b2[1:]
    delta = ffn - li
    Hm = H.reshape(NTOK, 4, 4)
    mixed = jnp.einsum('tij,tjc->tic', Hm, x4)
    return mixed + h_post[:, :, None] * delta[:, None, :]


def _sigmoid(z):
    return 0.5 * (1.0 + np.tanh(0.5 * z))


def _weights_key(ffn_w1, ffn_b1, ffn_w2, ffn_b2):
    h = 0
    for a in (ffn_w1, ffn_b1, ffn_w2, ffn_b2):
        a = np.asarray(a)
        s = np.ascontiguousarray(a.reshape(-1)[:: max(1, a.size // 4096)])
        h = zlib.adler32(s.tobytes(), h)
        h = zlib.adler32(np.float64(a.reshape(-1)[::997].sum()).tobytes(), h)
    return h


def _ensure_weights(ffn_w1, ffn_b1, ffn_w2, ffn_b2):
    """Quantize + pack weights and park them on device; returns (s1, s2)."""
    key = _weights_key(ffn_w1, ffn_b1, ffn_w2, ffn_b2)
    if _CACHE.get("wkey") == key:
        return _CACHE["ws1s2"]
    w1 = np.asarray(ffn_w1, np.float32)                    # [DFF, HID]
    w1t2 = np.ascontiguousarray(
        w1.reshape(32, 128, 8, 128).transpose(0, 3, 2, 1)
        .reshape(DFF, HID))                                # [m*128+p, k*128+j]
    s1 = float(np.abs(w1t2).max()) / 127.0
    w1q = np.clip(np.round(w1t2 / s1), -127, 127).astype(np.int8)
    w2t = np.ascontiguousarray(np.asarray(ffn_w2, np.float32).T)
    s2 = float(np.abs(w2t).max()) / 127.0
    w2q = np.clip(np.round(w2t / s2), -127, 127).astype(np.int8)
    b1r = np.ascontiguousarray(
        np.asarray(ffn_b1, np.float32).reshape(32, 128).T)  # [128, 32]

    wcore = np.empty((W_ROWS, 1024), np.int8)
    blobf = np.zeros((128, 64), np.float32)
    blobf[:, 0:32] = b1r
    wcore[0:DFF] = w1q
    wcore[DFF:2 * DFF] = w2q
    wcore[2 * DFF:2 * DFF + 32] = blobf.view(np.int8).reshape(32, 1024)
    r = get_runner()
    wdev = [jax.device_put(wcore, d) for d in r["devices"]]
    for w in wdev:
        w.block_until_ready()
    _CACHE["wdev"] = wdev
    _CACHE["wkey"] = key
    _CACHE["ws1s2"] = (s1, s2)
    return s1, s2


def _prep_full(x_streams, alpha_pre, alpha_post, alpha_res,
               W_pre_w, W_pre_b, W_post_w, W_post_b, W_res_w, W_res_b,
               ffn_w1, ffn_b1, ffn_w2, ffn_b2, perm_mat):
    s1, s2 = _ensure_weights(ffn_w1, ffn_b1, ffn_w2, ffn_b2)

    x = np.asarray(x_streams, np.float32).reshape(NTOK, NCH)
    wcat = np.concatenate([np.asarray(W_pre_w, np.float32),
                           np.asarray(W_post_w, np.float32),
                           np.asarray(W_res_w, np.float32)], axis=0).T
    bcat = np.concatenate([np.asarray(W_pre_b, np.float32),
                           np.asarray(W_post_b, np.float32),
                           np.asarray(W_res_b, np.float32)])
    a_pre = np.asarray(alpha_pre, np.float32).reshape(1)
    a_post = np.asarray(alpha_post, np.float32).reshape(1)
    a_res = np.asarray(alpha_res, np.float32).reshape(1)
    with jax.default_device(_CPU):
        li_q, rowmax, li, h_post, H = _prep_jit(
            x, wcat, bcat, a_pre, a_post, a_res,
            np.asarray(perm_mat, np.float32))
        li_q = np.asarray(li_q)
        rowmax = np.asarray(rowmax)
    lsc = ((rowmax / 127.0) * s1).astype(np.float32)       # fold s1

    li_global = np.empty((N_CORES * LI_ROWS, 1024), np.int8)
    for c in range(N_CORES):
        sl = slice(c * T_CORE, (c + 1) * T_CORE)
        blk = li_global[c * LI_ROWS:(c + 1) * LI_ROWS]
        blk[0:T_CORE] = li_q[sl]
        lscf = np.ascontiguousarray(lsc[sl].reshape(8, 128).T)  # [p, r]
        blk[T_CORE:T_CORE + 4] = lscf.view(np.int8).reshape(4, 1024)
    aux = dict(x4=x.reshape(NTOK, 4, HID), li=li, h_post=h_post, H=H,
               s2b2=np.concatenate([[np.float32(s2)],
                                    np.asarray(ffn_b2, np.float32)]))
    return li_global, aux


def _combine(y, aux):
    """y: [8, Y_ROWS, 1024] int8 device output -> full [4,2048,4,1024]."""
    q = y[:, 0:T_CORE, :].reshape(NTOK, 1024)
    ysc = np.ascontiguousarray(y[:, T_CORE:T_CORE + 4, :]) \
        .view(np.float32).reshape(NTOK, 1)
    with jax.default_device(_CPU):
        out = _combine_jit(q, ysc, aux["li"], aux["h_post"], aux["H"],
                           aux["x4"], aux["s2b2"])
        out = np.asarray(out)
    return out.reshape(4, 2048, 4, 1024)


def kernel(x_streams, alpha_pre, alpha_post, alpha_res,
           W_pre_w, W_pre_b, W_post_w, W_post_b, W_res_w, W_res_b,
           ffn_w1, ffn_b1, ffn_w2, ffn_b2, perm_mat):
    li_global, aux = _prep_full(x_streams, alpha_pre, alpha_post, alpha_res,
                                W_pre_w, W_pre_b, W_post_w, W_post_b,
                                W_res_w, W_res_b, ffn_w1, ffn_b1,
                                ffn_w2, ffn_b2, perm_mat)
    y = _run_device(li_global)
    return _combine(y, aux)


# revision 34
# speedup vs baseline: 1.2016x; 1.2016x over previous
"""MHC-lite block kernel for 8x TRN2 NeuronCores.

The wall-clock of a device call in this environment is dominated by
host<->device transfer over the axon tunnel (~43MB/s each direction,
partial duplex, ~80ms dispatch latency, ~50ms fixed cost per transfer),
so the design minimizes steady-state bytes on the wire:

  - host (f32): rms-norm stats, the three small projections (2 GFLOP),
    gates/softmax/H, layer_input (li), and the final mixed+expanded
    combine against the original f32 x.
  - device: FFN only (99% of model FLOPs), int8-in/int8-out.

Steady-state per-call traffic: 8.4MB li int8 h2d + 8.4MB y int8 d2h.
Everything else is off the per-call path:
  - FFN weights ship once (int8, 1/8 shard per core + on-device
    AllGather) and stay resident as a jax device array, keyed by a
    fingerprint of the incoming weights.
  - The donated output buffer is generated on-device (jnp.zeros under
    jit) instead of being transferred.
  - The PJRT wrapper (shard_map over _bass_exec_p) is jitted once and
    cached; per-call work is pure dispatch.

Quantization: w1,w2 int8 with global host scales s1,s2; s1 folds into
the per-token li dequant scale, s2 applies on host to the returned
per-token output scales. gelu at true scale, f32 PSUM; output re-quant
per token. f32 scales ride in the last rows of the int8 tensors via
bitcast APs.
"""

import numpy as np
import zlib

try:  # cache the XLA wrapper compile across processes
    import jax as _jax
    _jax.config.update("jax_compilation_cache_dir", "/tmp/jax_cache")
    _jax.config.update("jax_persistent_cache_min_compile_time_secs", 0.0)
    _jax.config.update("jax_persistent_cache_min_entry_size_bytes", -1)
except Exception:
    pass

import jax
import jax.numpy as jnp
from jax.sharding import Mesh, PartitionSpec, NamedSharding

import concourse.bacc as bacc
import concourse.mybir as mybir
import concourse.tile as tile
from concourse import bass2jax

try:
    from jax.experimental.shard_map import shard_map
except ImportError:
    from jax.shard_map import shard_map  # newer jax

N_CORES = 8
T_CORE = 1024          # tokens per core
NTOK = 8192            # total tokens
HID = 1024
NCH = 4096
DFF = 4096
EPS = 1.1920929e-07

# Wire precision: 6-bit planar-packed (4 values in 3 bytes) or plain int8.
LI_BITS = 6
OUT_BITS = 6
QL = float(2 ** (LI_BITS - 1) - 1)     # li quant max
QO = float(2 ** (OUT_BITS - 1) - 1)    # out quant max
LI_DATA = 768 if LI_BITS == 6 else 1024
Y_DATA = 768 if OUT_BITS == 6 else 1024
LI_COLS = LI_DATA + 4      # per-token row: packed data + f32 scale
Y_COLS = Y_DATA + 4
LI_ROWS = T_CORE
W_ROWS = 2 * DFF + 32      # full w1 (4096) + full w2 (4096) + 32 rows b1 f32
Y_ROWS = T_CORE

F32 = mybir.dt.float32
BF16 = mybir.dt.bfloat16
I8 = mybir.dt.int8
I32 = mybir.dt.int32
ALU = mybir.AluOpType

# planar permutation: device col h' = k*256 + j  <->  original index 4j + k
_hp = np.arange(1024)
_PLANAR = (4 * (_hp % 256) + _hp // 256).astype(np.int64)   # orig idx per h'
_INV_PLANAR = ((_hp % 4) * 256 + _hp // 4).astype(np.int64)  # h' per orig idx

_CACHE = {}


def _build_module():
    nc = bacc.Bacc("TRN2", target_bir_lowering=False, debug=False,
                   num_devices=N_CORES)
    li_d = nc.dram_tensor("li8", [LI_ROWS, LI_COLS], I8,
                          kind="ExternalInput").ap()
    wblob_d = nc.dram_tensor("wblob", [W_ROWS, 1024], I8,
                             kind="ExternalInput").ap()
    y_d = nc.dram_tensor("y8", [Y_ROWS, Y_COLS], I8,
                         kind="ExternalOutput").ap()

    with tile.TileContext(nc, trace_sim=False) as tc:
        _emit(nc, tc, li_d, wblob_d, y_d)
    nc.compile()
    return nc


def _emit(nc, tc, li_d, wblob_d, y_d):
    pools = []

    def _pool(*a, **k):
        p = tc.alloc_tile_pool(*a, **k)
        pools.append(p)
        return p

    # per-token trailing f32 scales
    lscv = li_d[:, LI_DATA:LI_DATA + 4].bitcast(F32)     # [1024, 1] f32
    yscv = y_d[:, Y_DATA:Y_DATA + 4].bitcast(F32)        # [1024, 1] f32

    # Full weights live in wblob (device-resident across calls; uploaded
    # once). No collective: keeps per-core executions independent.
    def _w1g_rows(m):                        # dff tile m of w1
        return wblob_d[m * 128:(m + 1) * 128, :]

    def _w2g_rows(m):                        # dff tile m of w2
        return wblob_d[DFF + m * 128:DFF + (m + 1) * 128, :]

    cp = _pool(name="const", bufs=1)
    w1_sb = cp.tile([128, 32 * HID], BF16, tag="w1sb")   # raw int values
    w2_sb = cp.tile([128, 32 * HID], BF16, tag="w2sb")   # raw int values
    f_sb = cp.tile([128, 64], F32, tag="fsb")            # b1 | pad
    lsc_sb = cp.tile([128, 8], F32, tag="lscsb")         # li scales, s1 folded
    lit_sb = cp.tile([128, 8 * T_CORE], BF16, tag="lit")  # li^T

    bview = wblob_d[2 * DFF:2 * DFF + 32, :].bitcast(F32) \
        .rearrange("a (p c) -> (a p) c", p=4)            # [128, 64] f32
    nc.sync.dma_start(f_sb[:, :], bview[:, :])
    b1_sb = f_sb[:, 0:32]
    for r in range(8):
        nc.sync.dma_start(lsc_sb[:, r:r + 1], lscv[r * 128:(r + 1) * 128, :])

    stp = _pool(name="stage", bufs=3)
    for m in range(32):
        st1 = stp.tile([128, HID], I8, tag="w1st")
        nc.sync.dma_start(st1[:, :], _w1g_rows(m))
        nc.vector.tensor_copy(w1_sb[:, m * HID:(m + 1) * HID], st1[:, :])
        st2 = stp.tile([128, HID], I8, tag="w2st")
        nc.sync.dma_start(st2[:, :], _w2g_rows(m))
        nc.vector.tensor_copy(w2_sb[:, m * HID:(m + 1) * HID], st2[:, :])

    # li: load packed token-major, unpack + dequant, transpose
    lqp = _pool(name="lq", bufs=3)
    lbp = _pool(name="lb", bufs=3)
    c32p = _pool(name="c32", bufs=2)
    tp = _pool(name="tmp6", bufs=2)
    for r in range(8):                       # token tiles
        rows = li_d[r * 128:(r + 1) * 128, 0:LI_DATA]
        libf = lbp.tile([128, HID], BF16, tag="libf")
        if LI_BITS == 8:
            li8 = lqp.tile([128, HID], I8, tag="li8")
            nc.sync.dma_start(li8[:, :], rows)
            nc.vector.tensor_scalar_mul(libf[:, :], li8[:, :],
                                        lsc_sb[:, r:r + 1])
        else:
            st = lqp.tile([128, LI_DATA], I8, tag="li6")
            nc.sync.dma_start(st[:, :], rows)
            cw = c32p.tile([128, LI_DATA], I32, tag="cw")
            nc.vector.tensor_copy(cw[:, :], st[:, :])
            c32 = c32p.tile([128, LI_DATA], I32, tag="c32")
            nc.vector.tensor_scalar_add(c32[:, :], cw[:, :], 128)
            c0, c1, c2 = (c32[:, 0:256], c32[:, 256:512], c32[:, 512:768])
            u = c32p.tile([128, HID], I32, tag="u6")
            # u0 = c0 & 63; u1 = (c0>>6) | ((c1&15)<<2)
            # u2 = (c1>>4) | ((c2&3)<<4); u3 = c2>>2
            nc.vector.tensor_scalar(u[:, 0:256], c0, 63, None,
                                    ALU.bitwise_and)
            ta = tp.tile([128, 256], I32, tag="t6a")
            nc.vector.tensor_scalar(ta[:, :], c0, 6, None,
                                    ALU.logical_shift_right)
            tb = tp.tile([128, 256], I32, tag="t6b")
            nc.vector.tensor_scalar(tb[:, :], c1, 15, None, ALU.bitwise_and)
            nc.vector.scalar_tensor_tensor(u[:, 256:512], tb[:, :], 4.0,
                                           ta[:, :], ALU.mult, ALU.add)
            tc_ = tp.tile([128, 256], I32, tag="t6c")
            nc.vector.tensor_scalar(tc_[:, :], c1, 4, None,
                                    ALU.logical_shift_right)
            td = tp.tile([128, 256], I32, tag="t6d")
            nc.vector.tensor_scalar(td[:, :], c2, 3, None, ALU.bitwise_and)
            nc.vector.scalar_tensor_tensor(u[:, 512:768], td[:, :], 16.0,
                                           tc_[:, :], ALU.mult, ALU.add)
            nc.vector.tensor_scalar(u[:, 768:1024], c2, 2, None,
                                    ALU.logical_shift_right)
            nc.vector.tensor_scalar(libf[:, :], u[:, :], -32.0,
                                    lsc_sb[:, r:r + 1], ALU.add, ALU.mult)
        for k in range(8):                   # hid tiles -> transpose
            nc.sync.dma_start_transpose(
                lit_sb[:, k * T_CORE + r * 128: k * T_CORE + r * 128 + 128],
                libf[:, k * 128:(k + 1) * 128])

    hp = _pool(name="h", bufs=4)
    qp = _pool(name="q", bufs=4)
    scp = _pool(name="sc", bufs=4)
    i32p = _pool(name="i32", bufs=2)
    pkp = _pool(name="pk", bufs=1)
    psA = _pool(name="psA", bufs=4, space="PSUM")
    psB = _pool(name="psB", bufs=4, space="PSUM")

    # mm2 for step m is emitted DELTA mm1-blocks later so the PE never
    # blocks on the just-issued gelu (~80us per blocking wait otherwise).
    DELTA = 3

    for g in range(4):                       # groups of 256 tokens
        t0 = g * 256
        fps = [[psA.tile([128, 512], F32, tag="psA",
                         name=f"fps_{g}_{ti}_{hf}")
                for hf in range(2)] for ti in range(2)]
        hs = {}

        def _mm2(m):
            h_src = hs.pop(m)
            for ti in range(2):
                for hf in range(2):
                    nc.tensor.matmul(
                        fps[ti][hf][:, :],
                        h_src[:, ti * 128:(ti + 1) * 128],
                        w2_sb[:, m * HID + hf * 512: m * HID + hf * 512 + 512],
                        start=(m == 0), stop=(m == 31))

        for m in range(32):                  # dff tiles of 128
            hmp = psB.tile([128, 256], F32, tag="psB")
            for k in range(8):               # hid contraction tiles
                nc.tensor.matmul(hmp[:, :],
                                 w1_sb[:, m * HID + k * 128:
                                       m * HID + k * 128 + 128],
                                 lit_sb[:, k * T_CORE + t0:
                                        k * T_CORE + t0 + 256],
                                 start=(k == 0), stop=(k == 7))
            h_m = hp.tile([128, 256], BF16, tag="h", name=f"h_{g}_{m}")
            nc.scalar.activation(h_m[:, :], hmp[:, :],
                                 mybir.ActivationFunctionType.Gelu_apprx_tanh,
                                 bias=b1_sb[:, m:m + 1])
            hs[m] = h_m
            if m >= DELTA:
                _mm2(m - DELTA)
        for m in range(32 - DELTA, 32):
            _mm2(m)
        for ti in range(2):
            # per-token absmax over the 1024 output cols (raw 1/s2 scale)
            sc = scp.tile([128, 8], F32, tag="sc")
            nc.vector.reduce_max(sc[:, 0:1], fps[ti][0][:, :],
                                 axis=mybir.AxisListType.X,
                                 apply_absolute_value=True)
            nc.vector.reduce_max(sc[:, 1:2], fps[ti][1][:, :],
                                 axis=mybir.AxisListType.X,
                                 apply_absolute_value=True)
            nc.vector.tensor_max(sc[:, 2:3], sc[:, 0:1], sc[:, 1:2])
            nc.scalar.activation(sc[:, 3:4], sc[:, 2:3],
                                 mybir.ActivationFunctionType.Copy,
                                 bias=1e-12)
            nc.vector.reciprocal(sc[:, 4:5], sc[:, 3:4])
            nc.scalar.activation(sc[:, 5:6], sc[:, 4:5],
                                 mybir.ActivationFunctionType.Copy,
                                 scale=QO)                     # inv
            nc.scalar.activation(sc[:, 6:7], sc[:, 3:4],
                                 mybir.ActivationFunctionType.Copy,
                                 scale=1.0 / QO)               # out scale
            trow = t0 + ti * 128
            nc.sync.dma_start(yscv[trow:trow + 128, :], sc[:, 6:7])
            if OUT_BITS == 8:
                for hf in range(2):
                    q = qp.tile([128, 512], I8, tag="q")
                    nc.vector.tensor_scalar_mul(q[:, :], fps[ti][hf][:, :],
                                                sc[:, 5:6])
                    nc.sync.dma_start(
                        y_d[trow:trow + 128, hf * 512:(hf + 1) * 512],
                        q[:, :])
            else:
                # v = round(y*inv) in [-31,31]; u = v+32; pack planar 4->3B.
                # Offsets are folded into the quant bias so every dual-op
                # stays (mult, add) and bitwise/shift ops are single-op.
                q0, q1 = fps[ti][0][:, 0:256], fps[ti][0][:, 256:512]
                q2, q3 = fps[ti][1][:, 0:256], fps[ti][1][:, 256:512]
                u1p = i32p.tile([128, 256], I32, tag="ua")   # u1 = v1+32
                nc.vector.tensor_scalar(u1p[:, :], q1, sc[:, 5:6], 32.0,
                                        ALU.mult, ALU.add)
                u2p = i32p.tile([128, 256], I32, tag="ub")   # u2 = v2+32
                nc.vector.tensor_scalar(u2p[:, :], q2, sc[:, 5:6], 32.0,
                                        ALU.mult, ALU.add)
                qt = qp.tile([128, Y_DATA], I8, tag="q6")
                # byte0 = (u1&3)*64 + (u0-128)
                tb0 = pkp.tile([128, 256], I32, tag="pk0")
                nc.vector.tensor_scalar(tb0[:, :], u1p[:, :], 3, None,
                                        ALU.bitwise_and)
                t1 = pkp.tile([128, 256], I32, tag="pk1")    # u0-128
                nc.vector.tensor_scalar(t1[:, :], q0, sc[:, 5:6], -96.0,
                                        ALU.mult, ALU.add)
                nc.vector.scalar_tensor_tensor(qt[:, 0:256], tb0[:, :], 64.0,
                                               t1[:, :], ALU.mult, ALU.add)
                # byte1 = (u2&15)*16 + ((u1>>2)-128) via (u1-512)>>2.
                # u1-512 must come from the SAME integer tensor as (u1&3)
                # above — a second independent round() can differ by 1 near
                # ties and corrupt the reassembled u1 by up to 4 quanta.
                t2s = pkp.tile([128, 256], I32, tag="pk2")   # u1-512
                nc.vector.tensor_scalar_add(t2s[:, :], u1p[:, :], -512)
                t2 = pkp.tile([128, 256], I32, tag="pk3")
                nc.vector.tensor_scalar(t2[:, :], t2s[:, :], 2, None,
                                        ALU.arith_shift_right)
                t3 = pkp.tile([128, 256], I32, tag="pk4")
                nc.vector.tensor_scalar(t3[:, :], u2p[:, :], 15, None,
                                        ALU.bitwise_and)
                nc.vector.scalar_tensor_tensor(qt[:, 256:512], t3[:, :],
                                               16.0, t2[:, :],
                                               ALU.mult, ALU.add)
                # byte2 = v3*4 + (u2>>4)  (== u3*4-128 + (u2>>4))
                t4 = pkp.tile([128, 256], I32, tag="pk5")
                nc.vector.tensor_scalar(t4[:, :], u2p[:, :], 4, None,
                                        ALU.logical_shift_right)
                t5 = pkp.tile([128, 256], I32, tag="pk6")    # v3
                nc.vector.tensor_scalar_mul(t5[:, :], q3, sc[:, 5:6])
                nc.vector.scalar_tensor_tensor(qt[:, 512:768], t5[:, :], 4.0,
                                               t4[:, :], ALU.mult, ALU.add)
                nc.sync.dma_start(y_d[trow:trow + 128, 0:Y_DATA], qt[:, :])

    for p in reversed(pools):
        p.release()


# ---------------------------------------------------------------------------
# PJRT runner: jitted once, reused every call.
# ---------------------------------------------------------------------------

def _build_runner(nc):
    bass2jax.install_neuronx_cc_hook()
    partition_name = nc.partition_id_tensor.name if nc.partition_id_tensor \
        else None
    in_names, out_names, out_avals, zero_shapes = [], [], [], []
    for alloc in nc.m.functions[0].allocations:
        if not isinstance(alloc, mybir.MemoryLocationSet):
            continue
        name = alloc.memorylocations[0].name
        if alloc.kind == "ExternalInput":
            if name != partition_name:
                in_names.append(name)
        elif alloc.kind == "ExternalOutput":
            shape = tuple(alloc.tensor_shape)
            dtype = mybir.dt.np(alloc.dtype)
            out_names.append(name)
            out_avals.append(jax.core.ShapedArray(shape, dtype))
            zero_shapes.append((shape, dtype))
    n_params = len(in_names)
    n_outs = len(out_avals)
    in_names_all = list(in_names) + list(out_names)
    if partition_name is not None:
        in_names_all.append(partition_name)

    def _body(*args):
        operands = list(args)
        if partition_name is not None:
            operands.append(bass2jax.partition_id_tensor())
        outs = bass2jax._bass_exec_p.bind(
            *operands, out_avals=tuple(out_avals),
            in_names=tuple(in_names_all), out_names=tuple(out_names),
            lowering_input_output_aliases=(), sim_require_finite=True,
            sim_require_nnan=True, nc=nc)
        return tuple(outs)

    devices = jax.devices()[:N_CORES]
    mesh = Mesh(np.asarray(devices), ("core",))
    P = PartitionSpec
    in_specs = (P("core"),) * (n_params + n_outs)
    out_specs = (P("core"),) * n_outs
    donate = tuple(range(n_params, n_params + n_outs))
    run = jax.jit(shard_map(_body, mesh=mesh, in_specs=in_specs,
                            out_specs=out_specs, check_rep=False),
                  donate_argnums=donate, keep_unused=True)
    shard = NamedSharding(mesh, P("core"))
    zshape, zdtype = zero_shapes[0]
    gz = (N_CORES * zshape[0],) + tuple(zshape[1:])
    mkzeros = jax.jit(lambda: jnp.zeros(gz, zdtype), out_shardings=shard)
    return dict(run=run, mkzeros=mkzeros, shard=shard, in_names=in_names,
                devices=devices)


def get_runner():
    if "runner" not in _CACHE:
        if "nc" not in _CACHE:
            _CACHE["nc"] = _build_module()
        _CACHE["runner"] = _build_runner(_CACHE["nc"])
    return _CACHE["runner"]


def _run_device(li_global):
    """li_global: [8*LI_ROWS, LI_COLS] int8 -> y: [8, Y_ROWS, Y_COLS] int8."""
    r = get_runner()
    xli = jax.device_put(li_global, r["shard"])
    z = r["mkzeros"]()
    (y,) = r["run"](xli, _CACHE["wdev"], z)
    return np.asarray(y).reshape(N_CORES, Y_ROWS, -1)


# ---------------------------------------------------------------------------
# Host-side prep (jax on the XLA CPU backend — numpy here links reference
# BLAS and is ~6x slower on the elementwise-heavy combine)
# ---------------------------------------------------------------------------

_CPU = jax.devices("cpu")[0]


@jax.jit
def _prep_jit(x, wcat, bcat, a_pre, a_post, a_res, perm):
    ssq = jnp.einsum('tc,tc->t', x, x)
    s = jax.lax.rsqrt(ssq / NCH + EPS)
    z = (x @ wcat) * s[:, None] + bcat                     # [NTOK, 32]
    h_pre = jax.nn.sigmoid(a_pre * z[:, 0:4])
    h_post = 2.0 * jax.nn.sigmoid(a_post * z[:, 4:8])
    a_soft = jax.nn.softmax(a_res * z[:, 8:32], axis=-1)
    H = a_soft @ perm                                      # [NTOK, 16]
    x4 = x.reshape(NTOK, 4, HID)
    li = jnp.einsum('tn,tnc->tc', h_pre, x4)
    rowmax = jnp.abs(li).max(axis=1) + 1e-30
    v = jnp.clip(jnp.round(li * (QL / rowmax[:, None])),
                 -QL, QL).astype(jnp.int32)
    if LI_BITS == 8:
        data = v.astype(jnp.int8)
    else:
        u = (v + 32).reshape(NTOK, 256, 4)
        u0, u1, u2, u3 = u[:, :, 0], u[:, :, 1], u[:, :, 2], u[:, :, 3]
        c0 = u0 | ((u1 & 3) << 6)
        c1 = (u1 >> 2) | ((u2 & 15) << 4)
        c2 = (u2 >> 4) | (u3 << 2)
        data = (jnp.concatenate([c0, c1, c2], axis=1) - 128).astype(jnp.int8)
    return data, rowmax, li, h_post, H


@jax.jit
def _combine_jit(y, li, h_post, H, x4, s2b2):
    ysc = jax.lax.bitcast_convert_type(
        y[:, Y_DATA:Y_DATA + 4], jnp.float32)[:, None]     # [NTOK, 1]
    if OUT_BITS == 8:
        ffn = y[:, 0:Y_DATA].astype(jnp.float32) * (ysc * s2b2[0]) + s2b2[1:]
    else:
        c = y[:, 0:Y_DATA].astype(jnp.int32) + 128
        c0, c1, c2 = c[:, 0:256], c[:, 256:512], c[:, 512:768]
        u0 = c0 & 63
        u1 = (c0 >> 6) | ((c1 & 15) << 2)
        u2 = (c1 >> 4) | ((c2 & 3) << 4)
        u3 = c2 >> 2
        vpl = jnp.concatenate([u0, u1, u2, u3], axis=1) - 32
        ffn_pl = vpl.astype(jnp.float32) * (ysc * s2b2[0])
        ffn = ffn_pl[:, _INV_PLANAR] + s2b2[1:]
    delta = ffn - li
    Hm = H.reshape(NTOK, 4, 4)
    mixed = jnp.einsum('tij,tjc->tic', Hm, x4)
    return mixed + h_post[:, :, None] * delta[:, None, :]


def _sigmoid(z):
    return 0.5 * (1.0 + np.tanh(0.5 * z))


def _weights_key(ffn_w1, ffn_b1, ffn_w2, ffn_b2):
    h = 0
    for a in (ffn_w1, ffn_b1, ffn_w2, ffn_b2):
        a = np.asarray(a)
        s = np.ascontiguousarray(a.reshape(-1)[:: max(1, a.size // 4096)])
        h = zlib.adler32(s.tobytes(), h)
        h = zlib.adler32(np.float64(a.reshape(-1)[::997].sum()).tobytes(), h)
    return h


def _ensure_weights(ffn_w1, ffn_b1, ffn_w2, ffn_b2):
    """Quantize + pack weights and park them on device; returns (s1, s2)."""
    key = _weights_key(ffn_w1, ffn_b1, ffn_w2, ffn_b2)
    if _CACHE.get("wkey") == key:
        return _CACHE["ws1s2"]
    w1 = np.asarray(ffn_w1, np.float32)                    # [DFF, HID]
    if LI_BITS == 6:
        w1 = w1[:, _PLANAR]          # planar hid order to match li unpack
    w1t2 = np.ascontiguousarray(
        w1.reshape(32, 128, 8, 128).transpose(0, 3, 2, 1)
        .reshape(DFF, HID))                                # [m*128+p, k*128+j]
    s1 = float(np.abs(w1t2).max()) / 127.0
    w1q = np.clip(np.round(w1t2 / s1), -127, 127).astype(np.int8)
    w2t = np.ascontiguousarray(np.asarray(ffn_w2, np.float32).T)
    if OUT_BITS == 6:
        w2t = w2t[:, _PLANAR]        # planar out-channel order for packing
    s2 = float(np.abs(w2t).max()) / 127.0
    w2q = np.clip(np.round(w2t / s2), -127, 127).astype(np.int8)
    b1r = np.ascontiguousarray(
        np.asarray(ffn_b1, np.float32).reshape(32, 128).T)  # [128, 32]

    wcore = np.empty((W_ROWS, 1024), np.int8)
    blobf = np.zeros((128, 64), np.float32)
    blobf[:, 0:32] = b1r
    wcore[0:DFF] = w1q
    wcore[DFF:2 * DFF] = w2q
    wcore[2 * DFF:2 * DFF + 32] = blobf.view(np.int8).reshape(32, 1024)
    r = get_runner()
    wg = np.broadcast_to(wcore, (N_CORES, W_ROWS, 1024)) \
        .reshape(N_CORES * W_ROWS, 1024)
    wdev = jax.device_put(wg, r["shard"])
    wdev.block_until_ready()
    _CACHE["wdev"] = wdev
    _CACHE["wkey"] = key
    _CACHE["ws1s2"] = (s1, s2)
    return s1, s2


def _prep_full(x_streams, alpha_pre, alpha_post, alpha_res,
               W_pre_w, W_pre_b, W_post_w, W_post_b, W_res_w, W_res_b,
               ffn_w1, ffn_b1, ffn_w2, ffn_b2, perm_mat):
    s1, s2 = _ensure_weights(ffn_w1, ffn_b1, ffn_w2, ffn_b2)

    x = np.asarray(x_streams, np.float32).reshape(NTOK, NCH)
    wcat = np.concatenate([np.asarray(W_pre_w, np.float32),
                           np.asarray(W_post_w, np.float32),
                           np.asarray(W_res_w, np.float32)], axis=0).T
    bcat = np.concatenate([np.asarray(W_pre_b, np.float32),
                           np.asarray(W_post_b, np.float32),
                           np.asarray(W_res_b, np.float32)])
    a_pre = np.asarray(alpha_pre, np.float32).reshape(1)
    a_post = np.asarray(alpha_post, np.float32).reshape(1)
    a_res = np.asarray(alpha_res, np.float32).reshape(1)
    with jax.default_device(_CPU):
        data, rowmax, li, h_post, H = _prep_jit(
            x, wcat, bcat, a_pre, a_post, a_res,
            np.asarray(perm_mat, np.float32))
        data = np.asarray(data)
        rowmax = np.asarray(rowmax)
    lsc = ((rowmax / QL) * s1).astype(np.float32)          # fold s1

    li_global = np.empty((NTOK, LI_COLS), np.int8)
    li_global[:, 0:LI_DATA] = data
    li_global[:, LI_DATA:] = lsc.view(np.int8).reshape(NTOK, 4)
    aux = dict(x4=x.reshape(NTOK, 4, HID), li=li, h_post=h_post, H=H,
               s2b2=np.concatenate([[np.float32(s2)],
                                    np.asarray(ffn_b2, np.float32)]))
    return li_global, aux


def _combine(y, aux):
    """y: [8, Y_ROWS, Y_COLS] int8 device output -> full [4,2048,4,1024]."""
    yf = y.reshape(NTOK, Y_COLS)
    with jax.default_device(_CPU):
        out = _combine_jit(yf, aux["li"], aux["h_post"], aux["H"],
                           aux["x4"], aux["s2b2"])
        out = np.asarray(out)
    return out.reshape(4, 2048, 4, 1024)


def kernel(x_streams, alpha_pre, alpha_post, alpha_res,
           W_pre_w, W_pre_b, W_post_w, W_post_b, W_res_w, W_res_b,
           ffn_w1, ffn_b1, ffn_w2, ffn_b2, perm_mat):
    li_global, aux = _prep_full(x_streams, alpha_pre, alpha_post, alpha_res,
                                W_pre_w, W_pre_b, W_post_w, W_post_b,
                                W_res_w, W_res_b, ffn_w1, ffn_b1,
                                ffn_w2, ffn_b2, perm_mat)
    y = _run_device(li_global)
    return _combine(y, aux)
